# revision 8
# baseline (speedup 1.0000x reference)
"""Causal self-attention (B=2, T=2048, C=1024, H=16) on 8 TRN2 NeuronCores.

Sharding: core = b*4 + g  (b in 0..1 batches, g in 0..3 head-groups of 4 heads).
Each core computes QKV for its 4 heads (tensor-parallel columns of W_attn),
full causal attention over T=2048, and a partial projection
y_g @ W_proj[rows_g] -> [T, C].  Host sums the 4 partials per batch and adds
b_proj.

Device layout (v2 — head-pair tile_position packing):
  - x is pre-transposed on host to xT [C, T]; all matmuls contract over
    partitions.  All matmul inputs bf16, accumulation f32.
  - Heads are processed in PAIRS (0,1) and (2,3) per query super-tile.
    sim^T = k.q is computed with K=64 row-tiled matmuls: head A at array
    rows 0-63, head B at rows 64-127 — the PE runs both CONCURRENTLY
    (disjoint row groups), halving sim wall time vs serial K=64 MMs.
  - exp via ScalarE (scale=1/sqrt(C) folded), trimmed to the causally valid
    column range; causal zeroing via gpsimd affine_select on diagonal tiles.
  - attn@v: col-tiled pair — head A -> yps[0:64], head B -> yps[64:128]
    (M=64 each, col groups 0-1 / 2-3, concurrent).  Softmax denominators
    via separate M=1 ones-matmuls into a shared PSUM tile, col-tiled
    pairwise (rows alternate {0,64}/{32,96} between successive pairs to
    avoid WAR stalls on the single denominator bank).
  - normalize: DVE reciprocal of the denominator rows, gpsimd
    partition_broadcast (DMA partition-hop when the row isn't physical
    partition 0), DVE multiply straight into yta — the pair layout already
    matches the projection's 2-heads-per-128-partitions packing, so the
    odd-head repack DMA of v1 is gone.
  - QKV / v / projection matmul groups are emitted as FILLER STEPS woven
    between attention j-iterations (software pipelining): during the
    ACT-paced attention phase the PE always has an independent GEMM to run.
  - PSUM budget: mm pool 1 bank, sim 2x[128,1024] = 4, yps 2, denom 1 = 8.
"""

import sys

sys.path.insert(0, "/opt/trn_rl_repo")

import numpy as np
import ml_dtypes

BF16 = ml_dtypes.bfloat16

B, T, C = 2, 2048, 1024
H, D = 16, 64
HPC = 4          # heads per core
GC = HPC * D     # head-group channel width (256)
NT = T // 128    # 16 row tiles
NS = T // 512    # 4 query super-tiles

_cached = None


def _build(repeat=1):
    import concourse.bass as bass  # noqa: F401
    import concourse.mybir as mybir
    import concourse.tile as tile
    from concourse import bacc

    f32 = mybir.dt.float32
    bf16 = mybir.dt.bfloat16
    AF = mybir.ActivationFunctionType

    nc = bacc.Bacc(None, target_bir_lowering=False, debug=False)
    xt_d = nc.declare_dram_parameter("xt", [C, T], bf16, isOutput=False)
    wqk_d = nc.declare_dram_parameter("wqk", [C, 2 * GC], bf16, isOutput=False)
    wv_d = nc.declare_dram_parameter("wv", [C, GC], bf16, isOutput=False)
    wp_d = nc.declare_dram_parameter("wp", [GC, C], bf16, isOutput=False)
    bqk_d = nc.declare_dram_parameter("bqk", [2 * GC], f32, isOutput=False)
    bv_d = nc.declare_dram_parameter("bv", [GC], f32, isOutput=False)
    out_d = nc.declare_dram_parameter("out", [T, C], bf16, isOutput=True)

    with tile.TileContext(nc) as tc:
        with (
            tc.tile_pool(name="const", bufs=1) as cpool,
            tc.tile_pool(name="exp", bufs=3) as epool,
            tc.tile_pool(name="ostg", bufs=2) as opool,
            tc.tile_pool(name="smal", bufs=3) as spool,
            tc.tile_pool(name="mm", bufs=1, space="PSUM") as mmp,
            tc.tile_pool(name="sim", bufs=2, space="PSUM") as simp,
            tc.tile_pool(name="yp", bufs=2, space="PSUM") as ypp,
            tc.tile_pool(name="dp", bufs=1, space="PSUM") as dpp,
        ):

            def emit_once():
                # ---- DMAs: first-needed first; wqk m-split so the first
                # QK group only gates on 128KB of weights + 1MB of x ----
                xt_re = xt_d[:].rearrange("(c p) t -> p c t", p=128)
                wqk_re = wqk_d[:].rearrange("(c p) m -> p c m", p=128)
                wqkm = []
                wqkm0 = cpool.tile([128, 8, 128], bf16, tag="wqkm0")
                wqkm.append(wqkm0)
                nc.sync.dma_start(out=wqkm[0][:], in_=wqk_re[:, :, 0:128])
                xa = cpool.tile([128, 4, 512], bf16, tag="xt0a")
                nc.sync.dma_start(out=xa[:], in_=xt_re[:, 0:4, 0:512])
                xb = cpool.tile([128, 4, 512], bf16, tag="xt0b")
                nc.sync.dma_start(out=xb[:], in_=xt_re[:, 4:8, 0:512])
                for m in range(1, 4):
                    wqkm_t = cpool.tile([128, 8, 128], bf16, tag=f"wqkm{m}")
                    nc.sync.dma_start(
                        out=wqkm_t[:], in_=wqk_re[:, :, m * 128 : (m + 1) * 128]
                    )
                    wqkm.append(wqkm_t)
                bqk = cpool.tile([128, 4], f32, tag="bqk")
                nc.sync.dma_start(
                    out=bqk[:], in_=bqk_d[:].rearrange("(m p) -> p m", p=128)
                )
                bv1 = cpool.tile([1, GC], f32, tag="bv1")
                nc.sync.dma_start(
                    out=bv1[:], in_=bv_d[:].rearrange("(o v) -> o v", o=1)
                )
                wv = cpool.tile([128, 8, GC], bf16, tag="wv")
                nc.sync.dma_start(
                    out=wv[:], in_=wv_d[:].rearrange("(c p) m -> p c m", p=128)
                )
                xt_parts = [[(xa, 0), (xb, 4)]]
                for s in range(1, NS):
                    x_s = cpool.tile([128, 8, 512], bf16, tag=f"xt{s}")
                    nc.sync.dma_start(
                        out=x_s[:], in_=xt_re[:, :, s * 512 : (s + 1) * 512]
                    )
                    xt_parts.append([(x_s, 0)])
                wp = cpool.tile([128, 2, C], bf16, tag="wp")
                nc.sync.dma_start(
                    out=wp[:], in_=wp_d[:].rearrange("(j p) n -> p j n", p=128)
                )

                def xslice(s, c):
                    for t_, c0 in xt_parts[s]:
                        if c0 <= c < c0 + 4 or (c0 == 0 and len(xt_parts[s]) == 1):
                            return t_[:, c - c0, :]
                    raise AssertionError

                ones = cpool.tile([1, 128], f32, tag="ones")
                nc.any.memset(ones[:], 1.0)
                onec = cpool.tile([128, 1], bf16, tag="onec")
                nc.any.memset(onec[:], 1.0)
                ones64 = cpool.tile([65, 64], f32, tag="ones64")
                nc.any.memset(ones64[64:65, :], 1.0)
                zbias = cpool.tile([128, 1], f32, tag="zbias")
                nc.any.memset(zbias[:], 0.0)

                qkT = cpool.tile([128, 4, T], bf16, tag="qkT")
                bvb = cpool.tile([128, GC], f32, tag="bvb")
                v1 = cpool.tile([128, NT, HPC, D], bf16, tag="v1")
                yta = cpool.tile([128, 2, T], bf16, tag="yta")
                dps = dpp.tile([128, 512], f32, tag="dps")

                # ---- filler step factories (each step ~2 matmuls or one
                # evict; emitted between attention iterations) ----
                def make_steps_qk(s):
                    steps = []
                    for m in range(4):
                        cell = {}

                        def s1(s=s, m=m, cell=cell):
                            ps = mmp.tile([128, 512], f32, tag="mm", name="mmq")
                            cell["ps"] = ps
                            for c in range(4):
                                nc.tensor.matmul(
                                    ps[:],
                                    wqkm[m][:, c, :],
                                    xslice(s, c),
                                    start=(c == 0),
                                    stop=False,
                                )

                        def s2(s=s, m=m, cell=cell):
                            ps = cell["ps"]
                            for c in range(4, 8):
                                nc.tensor.matmul(
                                    ps[:],
                                    wqkm[m][:, c, :],
                                    xslice(s, c),
                                    start=False,
                                    stop=(c == 7),
                                )

                        def s3(s=s, m=m, cell=cell):
                            nc.vector.tensor_scalar_add(
                                qkT[:, m, s * 512 : (s + 1) * 512],
                                cell["ps"][:],
                                bqk[:, m : m + 1],
                            )

                        steps += [s1, s2, s3]
                    return steps

                def make_steps_v(s):
                    steps = []
                    for t in range(s * 4, s * 4 + 4):
                        cell = {}

                        def s1(s=s, t=t, cell=cell):
                            ps = mmp.tile([128, GC], f32, tag="mm", name="mmv")
                            cell["ps"] = ps
                            for c in range(4):
                                nc.tensor.matmul(
                                    ps[:],
                                    xslice(s, c)[
                                        :, (t - 4 * s) * 128 : (t - 4 * s + 1) * 128
                                    ],
                                    wv[:, c, :],
                                    start=(c == 0),
                                    stop=False,
                                )

                        def s2(s=s, t=t, cell=cell):
                            ps = cell["ps"]
                            for c in range(4, 8):
                                nc.tensor.matmul(
                                    ps[:],
                                    xslice(s, c)[
                                        :, (t - 4 * s) * 128 : (t - 4 * s + 1) * 128
                                    ],
                                    wv[:, c, :],
                                    start=False,
                                    stop=(c == 7),
                                )

                        def s3(t=t, cell=cell):
                            nc.vector.tensor_add(
                                v1[:, t, :, :],
                                cell["ps"][:].rearrange("p (l d) -> p l d", d=D),
                                bvb[:].rearrange("p (l d) -> p l d", d=D),
                            )

                        steps += [s1, s2, s3]
                    return steps

                def make_steps_proj(s):
                    steps = []
                    for tt in range(4):
                        t = s * 4 + tt
                        cell = {}

                        def s0(cell=cell):
                            cell["ost"] = opool.tile([128, C], bf16, tag="ost", name="ost")

                        steps.append(s0)
                        for n in range(2):

                            def sA(t=t, n=n, cell=cell):
                                pp = mmp.tile([128, 512], f32, tag="mm", name="mmp")
                                cell["pp"] = pp
                                for j in range(2):
                                    nc.tensor.matmul(
                                        pp[:],
                                        yta[:, j, t * 128 : (t + 1) * 128],
                                        wp[:, j, n * 512 : (n + 1) * 512],
                                        start=(j == 0),
                                        stop=(j == 1),
                                    )

                            def sB(n=n, cell=cell):
                                nc.vector.tensor_copy(
                                    cell["ost"][:, n * 512 : (n + 1) * 512],
                                    cell["pp"][:],
                                )

                            steps += [sA, sB]

                        def sD(t=t, cell=cell):
                            nc.sync.dma_start(
                                out=out_d[t * 128 : (t + 1) * 128, :],
                                in_=cell["ost"][:],
                            )

                        steps.append(sD)
                    return steps

                # ---- attention for one head pair over one super-tile ----
                def emit_att(s, p, gidx, filler):
                    njt = 4 * (s + 1)
                    heads = (2 * p, 2 * p + 1)
                    rows = (0, 64) if (gidx % 2 == 1) else (32, 96)
                    yps = ypp.tile([128, 512], f32, tag="y")

                    def emit_sim(j):
                        q0 = (j - 4 * s) * 128 if j > 4 * s else 0
                        sp = simp.tile([128, 1024], f32, tag="sim")
                        for h in range(2):
                            po = h * 64
                            nc.tensor.matmul(
                                sp[:, h * 512 + q0 : (h + 1) * 512],
                                qkT[po : po + 64, 2 + p, j * 128 : (j + 1) * 128],
                                qkT[po : po + 64, p, s * 512 + q0 : (s + 1) * 512],
                                start=True,
                                stop=True,
                            )
                        return sp, q0

                    pend = emit_sim(0)
                    for _ in range(2):
                        if filler:
                            filler.pop(0)()
                    for j in range(njt):
                        sp, q0 = pend
                        if j + 1 < njt:
                            pend = emit_sim(j + 1)
                        ex = epool.tile([128, 1024], bf16, tag="exp")
                        if q0 == 0:
                            nc.scalar.activation(
                                ex[:],
                                sp[:],
                                AF.Exp,
                                bias=zbias[:, 0:1],
                                scale=1.0 / 32.0,
                            )
                        else:
                            # one act over both heads' valid ranges via a
                            # strided [128, 2, 512-q0] AP
                            ex3 = ex[:].rearrange("p (h w) -> p h w", h=2)
                            sp3 = sp[:].rearrange("p (h w) -> p h w", h=2)
                            nc.scalar.activation(
                                ex3[:, :, q0:],
                                sp3[:, :, q0:],
                                AF.Exp,
                                bias=zbias[:, 0:1],
                                scale=1.0 / 32.0,
                            )
                        r = j - 4 * s
                        if 0 <= r < 4:
                            for h in range(2):
                                nc.gpsimd.affine_select(
                                    out=ex[:, h * 512 + q0 : (h + 1) * 512],
                                    in_=ex[:, h * 512 + q0 : (h + 1) * 512],
                                    pattern=[[1, 512 - q0]],
                                    compare_op=mybir.AluOpType.is_ge,
                                    fill=0.0,
                                    base=q0 - r * 128,
                                    channel_multiplier=-1,
                                )
                        for h in range(2):
                            nc.tensor.matmul(
                                yps[h * 64 : (h + 1) * 64, q0:],
                                v1[:, j, heads[h], :],
                                ex[:, h * 512 + q0 : (h + 1) * 512],
                                start=(j == 0),
                                stop=(j == njt - 1),
                                skip_group_check=True,
                            )
                        for h in range(2):
                            nc.tensor.matmul(
                                dps[rows[h] : rows[h] + 1, q0:],
                                onec[:, 0:1],
                                ex[:, h * 512 + q0 : (h + 1) * 512],
                                start=(j == 0),
                                stop=(j == njt - 1),
                                skip_group_check=True,
                                tile_position=(0, rows[h]),
                            )
                        if filler:
                            filler.pop(0)()

                    # normalize into yta (pair layout == projection layout)
                    rt = spool.tile([128, 512], f32, tag="rt")
                    for row in rows:
                        nc.vector.reciprocal(
                            rt[row : row + 1, :], dps[row : row + 1, :]
                        )
                    last_pair = s == NS - 1 and p == 1
                    for h in range(2):
                        row = rows[h]
                        ysl = yta[h * 64 : (h + 1) * 64, p, s * 512 : (s + 1) * 512]
                        if last_pair and row != 0:
                            # tail: engine-local PE broadcast avoids the
                            # SBUF->SBUF DMA hop latency
                            bp = mmp.tile([64, 512], f32, tag="mm")
                            nc.tensor.matmul(
                                bp[:],
                                ones64[64:65, :],
                                rt[row : row + 1, :],
                                start=True,
                                stop=True,
                            )
                            nc.vector.tensor_copy(ysl, yps[h * 64 : (h + 1) * 64, :])
                            nc.vector.tensor_mul(ysl, ysl, bp[:])
                            continue
                        if row == 0:
                            src = rt[0:1, :]
                        else:
                            rt0 = spool.tile([1, 512], f32, tag=f"rt0{h}")
                            nc.sync.dma_start(out=rt0[:], in_=rt[row : row + 1, :])
                            src = rt0[:]
                        bps = spool.tile([64, 512], f32, tag=f"bps{h}")
                        nc.gpsimd.partition_broadcast(bps[:], src)
                        nc.vector.tensor_mul(
                            ysl, yps[h * 64 : (h + 1) * 64, :], bps[:]
                        )

                # ---- prologue: QKV for super-tile 0 + bv broadcast ----
                for st in make_steps_qk(0):
                    st()
                pbv = mmp.tile([128, GC], f32, tag="mm")
                nc.tensor.matmul(
                    pbv[:], ones[:, 0:128], bv1[:], start=True, stop=True
                )
                nc.vector.tensor_copy(bvb[:], pbv[:])
                for st in make_steps_v(0):
                    st()

                # ---- main pipeline ----
                for s in range(NS):
                    filler = []
                    if s + 1 < NS:
                        filler += make_steps_qk(s + 1)
                        filler += make_steps_v(s + 1)
                    if s >= 1:
                        filler += make_steps_proj(s - 1)
                    for p in (0, 1):
                        emit_att(s, p, s * 2 + p, filler)
                    while filler:
                        filler.pop(0)()
                for st in make_steps_proj(NS - 1):
                    st()

            for _rep in range(repeat):
                emit_once()

    nc.compile()
    return nc


def _get_nc():
    global _cached
    if _cached is None:
        _cached = _build()
    return _cached


def build_in_maps(inputs):
    x = np.asarray(inputs["x"], dtype=np.float32)
    W_attn = np.asarray(inputs["W_attn"], dtype=np.float32)
    b_attn = np.asarray(inputs["b_attn"], dtype=np.float32)
    W_proj = np.asarray(inputs["W_proj"], dtype=np.float32)

    in_maps = []
    for b in range(B):
        xT = np.ascontiguousarray(x[b].T).astype(BF16)
        for g in range(4):
            c0 = g * GC
            wq = W_attn[:, c0 : c0 + GC]
            wk = W_attn[:, C + c0 : C + c0 + GC]
            wqk = np.ascontiguousarray(np.concatenate([wq, wk], axis=1)).astype(BF16)
            wv = np.ascontiguousarray(
                W_attn[:, 2 * C + c0 : 2 * C + c0 + GC]
            ).astype(BF16)
            wp = np.ascontiguousarray(W_proj[c0 : c0 + GC, :]).astype(BF16)
            bqk = np.concatenate(
                [b_attn[c0 : c0 + GC], b_attn[C + c0 : C + c0 + GC]]
            ).astype(np.float32)
            bv = np.ascontiguousarray(
                b_attn[2 * C + c0 : 2 * C + c0 + GC]
            ).astype(np.float32)
            in_maps.append(
                {"xt": xT, "wqk": wqk, "wv": wv, "wp": wp, "bqk": bqk, "bv": bv}
            )
    return in_maps


def kernel(x, W_attn, b_attn, W_proj, b_proj):
    from concourse.bass_utils import run_bass_kernel_spmd

    b_proj = np.asarray(b_proj, dtype=np.float32)
    nc = _get_nc()
    in_maps = build_in_maps(
        {"x": x, "W_attn": W_attn, "b_attn": b_attn, "W_proj": W_proj}
    )
    res = run_bass_kernel_spmd(nc, in_maps, core_ids=list(range(8)))
    out = np.zeros((B, T, C), dtype=np.float32)
    for b in range(B):
        for g in range(4):
            out[b] += res.results[b * 4 + g]["out"].astype(np.float32)
        out[b] += b_proj
    return out


# revision 10
# speedup vs baseline: 1.0012x; 1.0012x over previous
"""Causal self-attention (B=2, T=2048, C=1024, H=16) on 8 TRN2 NeuronCores.

Sharding: core = b*4 + g  (b in 0..1 batches, g in 0..3 head-groups of 4 heads).
Each core computes QKV for its 4 heads (tensor-parallel columns of W_attn),
full causal attention over T=2048, and a partial projection
y_g @ W_proj[rows_g] -> [T, C].  Host sums the 4 partials per batch and adds
b_proj.

Device layout (v2 — head-pair tile_position packing):
  - x is pre-transposed on host to xT [C, T]; all matmuls contract over
    partitions.  All matmul inputs bf16, accumulation f32.
  - Heads are processed in PAIRS (0,1) and (2,3) per query super-tile.
    sim^T = k.q is computed with K=64 row-tiled matmuls: head A at array
    rows 0-63, head B at rows 64-127 — the PE runs both CONCURRENTLY
    (disjoint row groups), halving sim wall time vs serial K=64 MMs.
  - exp via ScalarE (scale=1/sqrt(C) folded), trimmed to the causally valid
    column range; causal zeroing via gpsimd affine_select on diagonal tiles.
  - attn@v: col-tiled pair — head A -> yps[0:64], head B -> yps[64:128]
    (M=64 each, col groups 0-1 / 2-3, concurrent).  Softmax denominators
    via separate M=1 ones-matmuls into a shared PSUM tile, col-tiled
    pairwise (rows alternate {0,64}/{32,96} between successive pairs to
    avoid WAR stalls on the single denominator bank).
  - normalize: DVE reciprocal of the denominator rows, gpsimd
    partition_broadcast (DMA partition-hop when the row isn't physical
    partition 0), DVE multiply straight into yta — the pair layout already
    matches the projection's 2-heads-per-128-partitions packing, so the
    odd-head repack DMA of v1 is gone.
  - QKV / v / projection matmul groups are emitted as FILLER STEPS woven
    between attention j-iterations (software pipelining): during the
    ACT-paced attention phase the PE always has an independent GEMM to run.
  - PSUM budget: mm pool 1 bank, sim 2x[128,1024] = 4, yps 2, denom 1 = 8.
"""

import sys

sys.path.insert(0, "/opt/trn_rl_repo")

import numpy as np
import ml_dtypes

BF16 = ml_dtypes.bfloat16

B, T, C = 2, 2048, 1024
H, D = 16, 64
HPC = 4          # heads per core
GC = HPC * D     # head-group channel width (256)
NT = T // 128    # 16 row tiles
NS = T // 512    # 4 query super-tiles

_cached = None


def _build(repeat=1, mode="full"):
    import concourse.bass as bass  # noqa: F401
    import concourse.mybir as mybir
    import concourse.tile as tile
    from concourse import bacc

    f32 = mybir.dt.float32
    bf16 = mybir.dt.bfloat16
    AF = mybir.ActivationFunctionType

    nc = bacc.Bacc(None, target_bir_lowering=False, debug=False)
    xt_d = nc.declare_dram_parameter("xt", [C, T], bf16, isOutput=False)
    wqk_d = nc.declare_dram_parameter("wqk", [C, 2 * GC], bf16, isOutput=False)
    wv_d = nc.declare_dram_parameter("wv", [C, GC], bf16, isOutput=False)
    wp_d = nc.declare_dram_parameter("wp", [GC, C], bf16, isOutput=False)
    bqk_d = nc.declare_dram_parameter("bqk", [2 * GC], f32, isOutput=False)
    bv_d = nc.declare_dram_parameter("bv", [GC], f32, isOutput=False)
    out_d = nc.declare_dram_parameter("out", [T, C], bf16, isOutput=True)

    with tile.TileContext(nc) as tc:
        with (
            tc.tile_pool(name="const", bufs=1) as cpool,
            tc.tile_pool(name="exp", bufs=3) as epool,
            tc.tile_pool(name="ostg", bufs=2) as opool,
            tc.tile_pool(name="smal", bufs=3) as spool,
            tc.tile_pool(name="mm", bufs=1, space="PSUM") as mmp,
            tc.tile_pool(name="sim", bufs=2, space="PSUM") as simp,
            tc.tile_pool(name="yp", bufs=2, space="PSUM") as ypp,
            tc.tile_pool(name="dp", bufs=1, space="PSUM") as dpp,
        ):

            def emit_once():
                # ---- DMAs: first-needed first; wqk m-split so the first
                # QK group only gates on 128KB of weights + 1MB of x ----
                xt_re = xt_d[:].rearrange("(c p) t -> p c t", p=128)
                wqk_re = wqk_d[:].rearrange("(c p) m -> p c m", p=128)
                wqkm = []
                wqkm0 = cpool.tile([128, 8, 128], bf16, tag="wqkm0")
                wqkm.append(wqkm0)
                nc.sync.dma_start(out=wqkm[0][:], in_=wqk_re[:, :, 0:128])
                xa = cpool.tile([128, 4, 512], bf16, tag="xt0a")
                nc.sync.dma_start(out=xa[:], in_=xt_re[:, 0:4, 0:512])
                xb = cpool.tile([128, 4, 512], bf16, tag="xt0b")
                nc.sync.dma_start(out=xb[:], in_=xt_re[:, 4:8, 0:512])
                for m in range(1, 4):
                    wqkm_t = cpool.tile([128, 8, 128], bf16, tag=f"wqkm{m}")
                    nc.sync.dma_start(
                        out=wqkm_t[:], in_=wqk_re[:, :, m * 128 : (m + 1) * 128]
                    )
                    wqkm.append(wqkm_t)
                bqk = cpool.tile([128, 4], f32, tag="bqk")
                nc.sync.dma_start(
                    out=bqk[:], in_=bqk_d[:].rearrange("(m p) -> p m", p=128)
                )
                bv1 = cpool.tile([1, GC], f32, tag="bv1")
                nc.sync.dma_start(
                    out=bv1[:], in_=bv_d[:].rearrange("(o v) -> o v", o=1)
                )
                wv = cpool.tile([128, 8, GC], bf16, tag="wv")
                nc.sync.dma_start(
                    out=wv[:], in_=wv_d[:].rearrange("(c p) m -> p c m", p=128)
                )
                xt_parts = [[(xa, 0), (xb, 4)]]
                for s in range(1, NS):
                    x_s = cpool.tile([128, 8, 512], bf16, tag=f"xt{s}")
                    nc.sync.dma_start(
                        out=x_s[:], in_=xt_re[:, :, s * 512 : (s + 1) * 512]
                    )
                    xt_parts.append([(x_s, 0)])
                wp = cpool.tile([128, 2, C], bf16, tag="wp")
                nc.sync.dma_start(
                    out=wp[:], in_=wp_d[:].rearrange("(j p) n -> p j n", p=128)
                )

                def xslice(s, c):
                    for t_, c0 in xt_parts[s]:
                        if c0 <= c < c0 + 4 or (c0 == 0 and len(xt_parts[s]) == 1):
                            return t_[:, c - c0, :]
                    raise AssertionError

                ones = cpool.tile([1, 128], f32, tag="ones")
                nc.any.memset(ones[:], 1.0)
                onesb = cpool.tile([128, D], bf16, tag="onesb")
                nc.any.memset(onesb[:], 1.0)
                zbias = cpool.tile([128, 1], f32, tag="zbias")
                nc.any.memset(zbias[:], 0.0)

                qkT = cpool.tile([128, 4, T], bf16, tag="qkT")
                bvb = cpool.tile([128, GC], f32, tag="bvb")
                v1 = cpool.tile([128, NT, HPC, D], bf16, tag="v1")
                yta = cpool.tile([128, 2, T], bf16, tag="yta")
                dps = dpp.tile([128, 512], f32, tag="dps")

                # ---- filler step factories (each step ~2 matmuls or one
                # evict; emitted between attention iterations) ----
                def make_steps_qk(s):
                    steps = []
                    for m in range(4):
                        cell = {}

                        def s1(s=s, m=m, cell=cell):
                            ps = mmp.tile([128, 512], f32, tag="mm", name="mmq")
                            cell["ps"] = ps
                            for c in range(4):
                                nc.tensor.matmul(
                                    ps[:],
                                    wqkm[m][:, c, :],
                                    xslice(s, c),
                                    start=(c == 0),
                                    stop=False,
                                )

                        def s2(s=s, m=m, cell=cell):
                            ps = cell["ps"]
                            for c in range(4, 8):
                                nc.tensor.matmul(
                                    ps[:],
                                    wqkm[m][:, c, :],
                                    xslice(s, c),
                                    start=False,
                                    stop=(c == 7),
                                )

                        def s3(s=s, m=m, cell=cell):
                            nc.vector.tensor_scalar_add(
                                qkT[:, m, s * 512 : (s + 1) * 512],
                                cell["ps"][:],
                                bqk[:, m : m + 1],
                            )

                        steps += [s1, s2, s3]
                    return steps

                def make_steps_v(s):
                    steps = []
                    for t in range(s * 4, s * 4 + 4):
                        cell = {}

                        def s1(s=s, t=t, cell=cell):
                            ps = mmp.tile([128, GC], f32, tag="mm", name="mmv")
                            cell["ps"] = ps
                            for c in range(4):
                                nc.tensor.matmul(
                                    ps[:],
                                    xslice(s, c)[
                                        :, (t - 4 * s) * 128 : (t - 4 * s + 1) * 128
                                    ],
                                    wv[:, c, :],
                                    start=(c == 0),
                                    stop=False,
                                )

                        def s2(s=s, t=t, cell=cell):
                            ps = cell["ps"]
                            for c in range(4, 8):
                                nc.tensor.matmul(
                                    ps[:],
                                    xslice(s, c)[
                                        :, (t - 4 * s) * 128 : (t - 4 * s + 1) * 128
                                    ],
                                    wv[:, c, :],
                                    start=False,
                                    stop=(c == 7),
                                )

                        def s3(t=t, cell=cell):
                            nc.vector.tensor_add(
                                v1[:, t, :, :],
                                cell["ps"][:].rearrange("p (l d) -> p l d", d=D),
                                bvb[:].rearrange("p (l d) -> p l d", d=D),
                            )

                        steps += [s1, s2, s3]
                    return steps

                def make_steps_proj(s):
                    steps = []
                    for tt in range(4):
                        t = s * 4 + tt
                        cell = {}

                        def s0(cell=cell):
                            cell["ost"] = opool.tile([128, C], bf16, tag="ost", name="ost")

                        steps.append(s0)
                        for n in range(2):

                            def sA(t=t, n=n, cell=cell):
                                pp = mmp.tile([128, 512], f32, tag="mm", name="mmp")
                                cell["pp"] = pp
                                for j in range(2):
                                    nc.tensor.matmul(
                                        pp[:],
                                        yta[:, j, t * 128 : (t + 1) * 128],
                                        wp[:, j, n * 512 : (n + 1) * 512],
                                        start=(j == 0),
                                        stop=(j == 1),
                                    )

                            def sB(n=n, cell=cell):
                                nc.vector.tensor_copy(
                                    cell["ost"][:, n * 512 : (n + 1) * 512],
                                    cell["pp"][:],
                                )

                            steps += [sA, sB]

                        def sD(t=t, cell=cell):
                            nc.sync.dma_start(
                                out=out_d[t * 128 : (t + 1) * 128, :],
                                in_=cell["ost"][:],
                            )

                        steps.append(sD)
                    return steps

                # ---- attention for one head pair over one super-tile ----
                def emit_att(s, p, gidx, filler):
                    njt = 4 * (s + 1)
                    heads = (2 * p, 2 * p + 1)
                    yps = ypp.tile([128, 512], f32, tag="y")

                    def emit_sim(j):
                        q0 = (j - 4 * s) * 128 if j > 4 * s else 0
                        sp = simp.tile([128, 1024], f32, tag="sim")
                        for h in range(2):
                            po = 0 if mode == "simser" else h * 64
                            nc.tensor.matmul(
                                sp[:, h * 512 + q0 : (h + 1) * 512],
                                qkT[po : po + 64, 2 + p, j * 128 : (j + 1) * 128],
                                qkT[po : po + 64, p, s * 512 + q0 : (s + 1) * 512],
                                start=True,
                                stop=True,
                            )
                        return sp, q0

                    pend = emit_sim(0)
                    for _ in range(2):
                        if filler:
                            filler.pop(0)()
                    for j in range(njt):
                        sp, q0 = pend
                        if j + 1 < njt:
                            pend = emit_sim(j + 1)
                        ex = epool.tile([128, 1024], bf16, tag="exp")
                        if q0 == 0:
                            nc.scalar.activation(
                                ex[:],
                                sp[:],
                                AF.Exp,
                                bias=zbias[:, 0:1],
                                scale=1.0 / 32.0,
                            )
                        else:
                            # one act over both heads' valid ranges via a
                            # strided [128, 2, 512-q0] AP
                            ex3 = ex[:].rearrange("p (h w) -> p h w", h=2)
                            sp3 = sp[:].rearrange("p (h w) -> p h w", h=2)
                            nc.scalar.activation(
                                ex3[:, :, q0:],
                                sp3[:, :, q0:],
                                AF.Exp,
                                bias=zbias[:, 0:1],
                                scale=1.0 / 32.0,
                            )
                        r = j - 4 * s
                        if 0 <= r < 4:
                            for h in range(2):
                                nc.gpsimd.affine_select(
                                    out=ex[:, h * 512 + q0 : (h + 1) * 512],
                                    in_=ex[:, h * 512 + q0 : (h + 1) * 512],
                                    pattern=[[1, 512 - q0]],
                                    compare_op=mybir.AluOpType.is_ge,
                                    fill=0.0,
                                    base=q0 - r * 128,
                                    channel_multiplier=-1,
                                )
                        for h in range(2):
                            nc.tensor.matmul(
                                yps[h * 64 : (h + 1) * 64, q0:],
                                v1[:, j, heads[h], :],
                                ex[:, h * 512 + q0 : (h + 1) * 512],
                                start=(j == 0),
                                stop=(j == njt - 1),
                                skip_group_check=True,
                            )
                        if mode != "nod":
                            for h in range(2):
                                nc.tensor.matmul(
                                    dps[h * 64 : (h + 1) * 64, q0:],
                                    onesb[:],
                                    ex[:, h * 512 + q0 : (h + 1) * 512],
                                    start=(j == 0),
                                    stop=(j == njt - 1),
                                    skip_group_check=True,
                                )
                        if filler:
                            filler.pop(0)()

                    # normalize into yta (pair layout == projection layout):
                    # dps rows h*64..h*64+63 all hold the same denominator
                    # (all-ones M=64 lhsT), so normalization is a plain
                    # elementwise multiply - no partition broadcast needed.
                    if mode in ("nod", "donly"):
                        for h in range(2):
                            nc.vector.tensor_copy(
                                yta[h * 64 : (h + 1) * 64, p, s * 512 : (s + 1) * 512],
                                yps[h * 64 : (h + 1) * 64, :],
                            )
                        return
                    dinv = spool.tile([128, 512], f32, tag="dinv")
                    for h in range(2):
                        nc.vector.reciprocal(
                            dinv[h * 64 : (h + 1) * 64, :],
                            dps[h * 64 : (h + 1) * 64, :],
                        )
                    for h in range(2):
                        nc.vector.tensor_mul(
                            yta[h * 64 : (h + 1) * 64, p, s * 512 : (s + 1) * 512],
                            yps[h * 64 : (h + 1) * 64, :],
                            dinv[h * 64 : (h + 1) * 64, :],
                        )

                # ---- prologue: QKV for super-tile 0 + bv broadcast ----
                for st in make_steps_qk(0):
                    st()
                pbv = mmp.tile([128, GC], f32, tag="mm")
                nc.tensor.matmul(
                    pbv[:], ones[:, 0:128], bv1[:], start=True, stop=True
                )
                nc.vector.tensor_copy(bvb[:], pbv[:])
                for st in make_steps_v(0):
                    st()

                # ---- main pipeline ----
                for s in range(NS):
                    filler = []
                    if s + 1 < NS:
                        filler += make_steps_qk(s + 1)
                        filler += make_steps_v(s + 1)
                    if s >= 1:
                        filler += make_steps_proj(s - 1)
                    for p in (0, 1):
                        emit_att(s, p, s * 2 + p, filler)
                    while filler:
                        filler.pop(0)()
                for st in make_steps_proj(NS - 1):
                    st()

            for _rep in range(repeat):
                emit_once()

    nc.compile()
    return nc


def _get_nc():
    global _cached
    if _cached is None:
        _cached = _build()
    return _cached


def build_in_maps(inputs):
    x = np.asarray(inputs["x"], dtype=np.float32)
    W_attn = np.asarray(inputs["W_attn"], dtype=np.float32)
    b_attn = np.asarray(inputs["b_attn"], dtype=np.float32)
    W_proj = np.asarray(inputs["W_proj"], dtype=np.float32)

    in_maps = []
    for b in range(B):
        xT = np.ascontiguousarray(x[b].T).astype(BF16)
        for g in range(4):
            c0 = g * GC
            wq = W_attn[:, c0 : c0 + GC]
            wk = W_attn[:, C + c0 : C + c0 + GC]
            wqk = np.ascontiguousarray(np.concatenate([wq, wk], axis=1)).astype(BF16)
            wv = np.ascontiguousarray(
                W_attn[:, 2 * C + c0 : 2 * C + c0 + GC]
            ).astype(BF16)
            wp = np.ascontiguousarray(W_proj[c0 : c0 + GC, :]).astype(BF16)
            bqk = np.concatenate(
                [b_attn[c0 : c0 + GC], b_attn[C + c0 : C + c0 + GC]]
            ).astype(np.float32)
            bv = np.ascontiguousarray(
                b_attn[2 * C + c0 : 2 * C + c0 + GC]
            ).astype(np.float32)
            in_maps.append(
                {"xt": xT, "wqk": wqk, "wv": wv, "wp": wp, "bqk": bqk, "bv": bv}
            )
    return in_maps


def kernel(x, W_attn, b_attn, W_proj, b_proj):
    from concourse.bass_utils import run_bass_kernel_spmd

    b_proj = np.asarray(b_proj, dtype=np.float32)
    nc = _get_nc()
    in_maps = build_in_maps(
        {"x": x, "W_attn": W_attn, "b_attn": b_attn, "W_proj": W_proj}
    )
    res = run_bass_kernel_spmd(nc, in_maps, core_ids=list(range(8)))
    out = np.zeros((B, T, C), dtype=np.float32)
    for b in range(B):
        for g in range(4):
            out[b] += res.results[b * 4 + g]["out"].astype(np.float32)
        out[b] += b_proj
    return out


# revision 12
# speedup vs baseline: 1.2637x; 1.2622x over previous
"""Causal self-attention (B=2, T=2048, C=1024, H=16) on 8 TRN2 NeuronCores.

Sharding: core = b*4 + g  (b in 0..1 batches, g in 0..3 head-groups of 4 heads).
Each core computes QKV for its 4 heads (tensor-parallel columns of W_attn),
full causal attention over T=2048, and a partial projection
y_g @ W_proj[rows_g] -> [T, C].  Host sums the 4 partials per batch and adds
b_proj.

Device layout (v2 — head-pair tile_position packing):
  - x is pre-transposed on host to xT [C, T]; all matmuls contract over
    partitions.  All matmul inputs bf16, accumulation f32.
  - Heads are processed in PAIRS (0,1) and (2,3) per query super-tile.
    sim^T = k.q is computed with K=64 row-tiled matmuls: head A at array
    rows 0-63, head B at rows 64-127 — the PE runs both CONCURRENTLY
    (disjoint row groups), halving sim wall time vs serial K=64 MMs.
  - exp via ScalarE (scale=1/sqrt(C) folded), trimmed to the causally valid
    column range; causal zeroing via gpsimd affine_select on diagonal tiles.
  - attn@v: col-tiled pair — head A -> yps[0:64], head B -> yps[64:128]
    (M=64 each, col groups 0-1 / 2-3, concurrent).  Softmax denominators
    via separate M=1 ones-matmuls into a shared PSUM tile, col-tiled
    pairwise (rows alternate {0,64}/{32,96} between successive pairs to
    avoid WAR stalls on the single denominator bank).
  - normalize: DVE reciprocal of the denominator rows, gpsimd
    partition_broadcast (DMA partition-hop when the row isn't physical
    partition 0), DVE multiply straight into yta — the pair layout already
    matches the projection's 2-heads-per-128-partitions packing, so the
    odd-head repack DMA of v1 is gone.
  - QKV / v / projection matmul groups are emitted as FILLER STEPS woven
    between attention j-iterations (software pipelining): during the
    ACT-paced attention phase the PE always has an independent GEMM to run.
  - PSUM budget: mm pool 1 bank, sim 2x[128,1024] = 4, yps 2, denom 1 = 8.
"""

import sys

sys.path.insert(0, "/opt/trn_rl_repo")

import numpy as np
import ml_dtypes

BF16 = ml_dtypes.bfloat16

B, T, C = 2, 2048, 1024
H, D = 16, 64
HPC = 4          # heads per core
GC = HPC * D     # head-group channel width (256)
NT = T // 128    # 16 row tiles
NS = T // 512    # 4 query super-tiles

_cached = None


def _build(repeat=1, mode="full"):
    import concourse.bass as bass  # noqa: F401
    import concourse.mybir as mybir
    import concourse.tile as tile
    from concourse import bacc

    f32 = mybir.dt.float32
    bf16 = mybir.dt.bfloat16
    AF = mybir.ActivationFunctionType

    nc = bacc.Bacc(None, target_bir_lowering=False, debug=False)
    xt_d = nc.declare_dram_parameter("xt", [C, T], bf16, isOutput=False)
    wqk_d = nc.declare_dram_parameter("wqk", [C, 2 * GC], bf16, isOutput=False)
    wv_d = nc.declare_dram_parameter("wv", [C, GC], bf16, isOutput=False)
    wp_d = nc.declare_dram_parameter("wp", [GC, C], bf16, isOutput=False)
    bqk_d = nc.declare_dram_parameter("bqk", [2 * GC], f32, isOutput=False)
    bv_d = nc.declare_dram_parameter("bv", [GC], f32, isOutput=False)
    out_d = nc.declare_dram_parameter("out", [T, C], bf16, isOutput=True)

    with tile.TileContext(nc) as tc:
        with (
            tc.tile_pool(name="const", bufs=1) as cpool,
            tc.tile_pool(name="exp", bufs=3) as epool,
            tc.tile_pool(name="ostg", bufs=2) as opool,
            tc.tile_pool(name="smal", bufs=3) as spool,
            tc.tile_pool(name="mm", bufs=1, space="PSUM") as mmp,
            tc.tile_pool(name="sim", bufs=2, space="PSUM") as simp,
            tc.tile_pool(name="yp", bufs=2, space="PSUM") as ypp,
            tc.tile_pool(name="dp", bufs=1, space="PSUM") as dpp,
        ):

            def emit_once():
                # ---- DMAs: first-needed first; wqk m-split so the first
                # QK group only gates on 128KB of weights + 1MB of x ----
                xt_re = xt_d[:].rearrange("(c p) t -> p c t", p=128)
                wqk_re = wqk_d[:].rearrange("(c p) m -> p c m", p=128)
                wqkm = []
                wqkm0 = cpool.tile([128, 8, 128], bf16, tag="wqkm0")
                wqkm.append(wqkm0)
                nc.sync.dma_start(out=wqkm[0][:], in_=wqk_re[:, :, 0:128])
                xa = cpool.tile([128, 4, 512], bf16, tag="xt0a")
                nc.sync.dma_start(out=xa[:], in_=xt_re[:, 0:4, 0:512])
                xb = cpool.tile([128, 4, 512], bf16, tag="xt0b")
                nc.sync.dma_start(out=xb[:], in_=xt_re[:, 4:8, 0:512])
                for m in range(1, 4):
                    wqkm_t = cpool.tile([128, 8, 128], bf16, tag=f"wqkm{m}")
                    nc.sync.dma_start(
                        out=wqkm_t[:], in_=wqk_re[:, :, m * 128 : (m + 1) * 128]
                    )
                    wqkm.append(wqkm_t)
                bqk = cpool.tile([128, 4], f32, tag="bqk")
                nc.sync.dma_start(
                    out=bqk[:], in_=bqk_d[:].rearrange("(m p) -> p m", p=128)
                )
                bv1 = cpool.tile([1, GC], f32, tag="bv1")
                nc.sync.dma_start(
                    out=bv1[:], in_=bv_d[:].rearrange("(o v) -> o v", o=1)
                )
                wv = cpool.tile([128, 8, GC], bf16, tag="wv")
                nc.sync.dma_start(
                    out=wv[:], in_=wv_d[:].rearrange("(c p) m -> p c m", p=128)
                )
                xt_parts = [[(xa, 0), (xb, 4)]]
                for s in range(1, NS):
                    x_s = cpool.tile([128, 8, 512], bf16, tag=f"xt{s}")
                    nc.sync.dma_start(
                        out=x_s[:], in_=xt_re[:, :, s * 512 : (s + 1) * 512]
                    )
                    xt_parts.append([(x_s, 0)])
                wp = cpool.tile([128, 2, C], bf16, tag="wp")
                nc.sync.dma_start(
                    out=wp[:], in_=wp_d[:].rearrange("(j p) n -> p j n", p=128)
                )

                def xslice(s, c):
                    for t_, c0 in xt_parts[s]:
                        if c0 <= c < c0 + 4 or (c0 == 0 and len(xt_parts[s]) == 1):
                            return t_[:, c - c0, :]
                    raise AssertionError

                ones = cpool.tile([1, 128], f32, tag="ones")
                nc.any.memset(ones[:], 1.0)
                onesb = cpool.tile([128, D], bf16, tag="onesb")
                nc.any.memset(onesb[:], 1.0)
                zbias = cpool.tile([128, 1], f32, tag="zbias")
                nc.any.memset(zbias[:], 0.0)

                qkT = cpool.tile([128, 4, T], bf16, tag="qkT")
                bvb = cpool.tile([128, GC], f32, tag="bvb")
                v1 = cpool.tile([128, NT, HPC, D], bf16, tag="v1")
                yta = cpool.tile([128, 2, T], bf16, tag="yta")
                dps = dpp.tile([128, 512], f32, tag="dps")

                # ---- filler step factories (each step ~2 matmuls or one
                # evict; emitted between attention iterations) ----
                def make_steps_qk(s):
                    steps = []
                    for m in range(4):
                        cell = {}

                        def s1(s=s, m=m, cell=cell):
                            ps = mmp.tile([128, 512], f32, tag="mm", name="mmq")
                            cell["ps"] = ps
                            for c in range(4):
                                nc.tensor.matmul(
                                    ps[:],
                                    wqkm[m][:, c, :],
                                    xslice(s, c),
                                    start=(c == 0),
                                    stop=False,
                                )

                        def s2(s=s, m=m, cell=cell):
                            ps = cell["ps"]
                            for c in range(4, 8):
                                nc.tensor.matmul(
                                    ps[:],
                                    wqkm[m][:, c, :],
                                    xslice(s, c),
                                    start=False,
                                    stop=(c == 7),
                                )

                        def s3(s=s, m=m, cell=cell):
                            nc.vector.tensor_scalar_add(
                                qkT[:, m, s * 512 : (s + 1) * 512],
                                cell["ps"][:],
                                bqk[:, m : m + 1],
                            )

                        steps += [s1, s2, s3]
                    return steps

                def make_steps_v(s):
                    steps = []
                    for t in range(s * 4, s * 4 + 4):
                        cell = {}

                        def s1(s=s, t=t, cell=cell):
                            ps = mmp.tile([128, GC], f32, tag="mm", name="mmv")
                            cell["ps"] = ps
                            for c in range(4):
                                nc.tensor.matmul(
                                    ps[:],
                                    xslice(s, c)[
                                        :, (t - 4 * s) * 128 : (t - 4 * s + 1) * 128
                                    ],
                                    wv[:, c, :],
                                    start=(c == 0),
                                    stop=False,
                                )

                        def s2(s=s, t=t, cell=cell):
                            ps = cell["ps"]
                            for c in range(4, 8):
                                nc.tensor.matmul(
                                    ps[:],
                                    xslice(s, c)[
                                        :, (t - 4 * s) * 128 : (t - 4 * s + 1) * 128
                                    ],
                                    wv[:, c, :],
                                    start=False,
                                    stop=(c == 7),
                                )

                        def s3(t=t, cell=cell):
                            nc.vector.tensor_add(
                                v1[:, t, :, :],
                                cell["ps"][:].rearrange("p (l d) -> p l d", d=D),
                                bvb[:].rearrange("p (l d) -> p l d", d=D),
                            )

                        steps += [s1, s2, s3]
                    return steps

                def make_steps_proj(s):
                    steps = []
                    for tt in range(4):
                        t = s * 4 + tt
                        cell = {}

                        def s0(cell=cell):
                            cell["ost"] = opool.tile([128, C], bf16, tag="ost", name="ost")

                        steps.append(s0)
                        for n in range(2):

                            def sA(t=t, n=n, cell=cell):
                                pp = mmp.tile([128, 512], f32, tag="mm", name="mmp")
                                cell["pp"] = pp
                                for j in range(2):
                                    nc.tensor.matmul(
                                        pp[:],
                                        yta[:, j, t * 128 : (t + 1) * 128],
                                        wp[:, j, n * 512 : (n + 1) * 512],
                                        start=(j == 0),
                                        stop=(j == 1),
                                    )

                            def sB(n=n, cell=cell):
                                nc.vector.tensor_copy(
                                    cell["ost"][:, n * 512 : (n + 1) * 512],
                                    cell["pp"][:],
                                )

                            steps += [sA, sB]

                        def sD(t=t, cell=cell):
                            nc.sync.dma_start(
                                out=out_d[t * 128 : (t + 1) * 128, :],
                                in_=cell["ost"][:],
                            )

                        steps.append(sD)
                    return steps

                # ---- attention for one head pair over one super-tile ----
                def emit_att(s, p, gidx, filler):
                    njt = 4 * (s + 1)
                    heads = (2 * p, 2 * p + 1)
                    yps = ypp.tile([128, 512], f32, tag="y")

                    def emit_sim(j):
                        q0 = (j - 4 * s) * 128 if j > 4 * s else 0
                        sp = simp.tile([128, 1024], f32, tag="sim")
                        for h in range(2):
                            po = 0 if mode == "simser" else h * 64
                            nc.tensor.matmul(
                                sp[:, h * 512 + q0 : (h + 1) * 512],
                                qkT[po : po + 64, 2 + p, j * 128 : (j + 1) * 128],
                                qkT[po : po + 64, p, s * 512 + q0 : (s + 1) * 512],
                                start=True,
                                stop=True,
                            )
                        return sp, q0

                    pend = emit_sim(0)
                    for _ in range(2):
                        if filler:
                            filler.pop(0)()
                    for j in range(njt):
                        sp, q0 = pend
                        if j + 1 < njt:
                            pend = emit_sim(j + 1)
                        ex = epool.tile([128, 1024], bf16, tag="exp")
                        if q0 == 0:
                            nc.scalar.activation(
                                ex[:],
                                sp[:],
                                AF.Exp,
                                bias=zbias[:, 0:1],
                                scale=1.0 / 32.0,
                            )
                        else:
                            # one act over both heads' valid ranges via a
                            # strided [128, 2, 512-q0] AP
                            ex3 = ex[:].rearrange("p (h w) -> p h w", h=2)
                            sp3 = sp[:].rearrange("p (h w) -> p h w", h=2)
                            nc.scalar.activation(
                                ex3[:, :, q0:],
                                sp3[:, :, q0:],
                                AF.Exp,
                                bias=zbias[:, 0:1],
                                scale=1.0 / 32.0,
                            )
                        r = j - 4 * s
                        if 0 <= r < 4:
                            for h in range(2):
                                nc.gpsimd.affine_select(
                                    out=ex[:, h * 512 + q0 : (h + 1) * 512],
                                    in_=ex[:, h * 512 + q0 : (h + 1) * 512],
                                    pattern=[[1, 512 - q0]],
                                    compare_op=mybir.AluOpType.is_ge,
                                    fill=0.0,
                                    base=q0 - r * 128,
                                    channel_multiplier=-1,
                                )
                        for h in range(2):
                            nc.tensor.matmul(
                                yps[h * 64 : (h + 1) * 64, q0:],
                                v1[:, j, heads[h], :],
                                ex[:, h * 512 + q0 : (h + 1) * 512],
                                start=(j == 0),
                                stop=(j == njt - 1),
                                skip_group_check=True,
                            )
                        if mode != "nod":
                            for h in range(2):
                                nc.tensor.matmul(
                                    dps[h * 64 : (h + 1) * 64, q0:],
                                    onesb[:],
                                    ex[:, h * 512 + q0 : (h + 1) * 512],
                                    start=(j == 0),
                                    stop=(j == njt - 1),
                                    skip_group_check=True,
                                )
                        if filler:
                            filler.pop(0)()

                    # normalize into yta (pair layout == projection layout):
                    # dps rows h*64..h*64+63 all hold the same denominator
                    # (all-ones M=64 lhsT), so normalization is a plain
                    # elementwise multiply - no partition broadcast needed.
                    if mode in ("nod", "donly"):
                        for h in range(2):
                            nc.vector.tensor_copy(
                                yta[h * 64 : (h + 1) * 64, p, s * 512 : (s + 1) * 512],
                                yps[h * 64 : (h + 1) * 64, :],
                            )
                        return
                    # NOTE: reciprocal_approx_fast misbehaves at base
                    # partition 64, so run one op over all 128 partitions
                    # (every dps row holds a valid denominator).
                    dinv = spool.tile([128, 512], f32, tag="dinv")
                    nc.vector.reciprocal_approx_fast(dinv[:], dps[:])
                    for h in range(2):
                        nc.vector.tensor_mul(
                            yta[h * 64 : (h + 1) * 64, p, s * 512 : (s + 1) * 512],
                            yps[h * 64 : (h + 1) * 64, :],
                            dinv[h * 64 : (h + 1) * 64, :],
                        )

                # ---- prologue: QKV for super-tile 0 + bv broadcast ----
                for st in make_steps_qk(0):
                    st()
                pbv = mmp.tile([128, GC], f32, tag="mm")
                nc.tensor.matmul(
                    pbv[:], ones[:, 0:128], bv1[:], start=True, stop=True
                )
                nc.vector.tensor_copy(bvb[:], pbv[:])
                for st in make_steps_v(0):
                    st()

                # ---- main pipeline ----
                for s in range(NS):
                    filler = []
                    if s + 1 < NS:
                        filler += make_steps_qk(s + 1)
                        filler += make_steps_v(s + 1)
                    if s >= 1:
                        filler += make_steps_proj(s - 1)
                    for p in (0, 1):
                        emit_att(s, p, s * 2 + p, filler)
                    while filler:
                        filler.pop(0)()
                for st in make_steps_proj(NS - 1):
                    st()

            for _rep in range(repeat):
                emit_once()

    nc.compile()
    return nc


def _get_nc():
    global _cached
    if _cached is None:
        _cached = _build()
    return _cached


def build_in_maps(inputs):
    x = np.asarray(inputs["x"], dtype=np.float32)
    W_attn = np.asarray(inputs["W_attn"], dtype=np.float32)
    b_attn = np.asarray(inputs["b_attn"], dtype=np.float32)
    W_proj = np.asarray(inputs["W_proj"], dtype=np.float32)

    in_maps = []
    for b in range(B):
        xT = np.ascontiguousarray(x[b].T).astype(BF16)
        for g in range(4):
            c0 = g * GC
            wq = W_attn[:, c0 : c0 + GC]
            wk = W_attn[:, C + c0 : C + c0 + GC]
            wqk = np.ascontiguousarray(np.concatenate([wq, wk], axis=1)).astype(BF16)
            wv = np.ascontiguousarray(
                W_attn[:, 2 * C + c0 : 2 * C + c0 + GC]
            ).astype(BF16)
            wp = np.ascontiguousarray(W_proj[c0 : c0 + GC, :]).astype(BF16)
            bqk = np.concatenate(
                [b_attn[c0 : c0 + GC], b_attn[C + c0 : C + c0 + GC]]
            ).astype(np.float32)
            bv = np.ascontiguousarray(
                b_attn[2 * C + c0 : 2 * C + c0 + GC]
            ).astype(np.float32)
            in_maps.append(
                {"xt": xT, "wqk": wqk, "wv": wv, "wp": wp, "bqk": bqk, "bv": bv}
            )
    return in_maps


def kernel(x, W_attn, b_attn, W_proj, b_proj):
    from concourse.bass_utils import run_bass_kernel_spmd

    b_proj = np.asarray(b_proj, dtype=np.float32)
    nc = _get_nc()
    in_maps = build_in_maps(
        {"x": x, "W_attn": W_attn, "b_attn": b_attn, "W_proj": W_proj}
    )
    res = run_bass_kernel_spmd(nc, in_maps, core_ids=list(range(8)))
    out = np.zeros((B, T, C), dtype=np.float32)
    for b in range(B):
        for g in range(4):
            out[b] += res.results[b * 4 + g]["out"].astype(np.float32)
        out[b] += b_proj
    return out


# revision 15
# speedup vs baseline: 1.5039x; 1.1901x over previous
"""Causal self-attention (B=2, T=2048, C=1024, H=16) on 8 TRN2 NeuronCores.

Sharding: core = b*4 + g  (b in 0..1 batches, g in 0..3 head-groups of 4 heads).
Each core computes QKV for its 4 heads (tensor-parallel columns of W_attn),
full causal attention over T=2048, and a partial projection
y_g @ W_proj[rows_g] -> [T, C].  Host sums the 4 partials per batch and adds
b_proj.

Device layout (v2 — head-pair tile_position packing):
  - x is pre-transposed on host to xT [C, T]; all matmuls contract over
    partitions.  All matmul inputs bf16, accumulation f32.
  - Heads are processed in PAIRS (0,1) and (2,3) per query super-tile.
    sim^T = k.q is computed with K=64 row-tiled matmuls: head A at array
    rows 0-63, head B at rows 64-127 — the PE runs both CONCURRENTLY
    (disjoint row groups), halving sim wall time vs serial K=64 MMs.
  - exp via ScalarE (scale=1/sqrt(C) folded), trimmed to the causally valid
    column range; causal zeroing via gpsimd affine_select on diagonal tiles.
  - attn@v: col-tiled pair — head A -> yps[0:64], head B -> yps[64:128]
    (M=64 each, col groups 0-1 / 2-3, concurrent).  Softmax denominators
    via separate M=1 ones-matmuls into a shared PSUM tile, col-tiled
    pairwise (rows alternate {0,64}/{32,96} between successive pairs to
    avoid WAR stalls on the single denominator bank).
  - normalize: DVE reciprocal of the denominator rows, gpsimd
    partition_broadcast (DMA partition-hop when the row isn't physical
    partition 0), DVE multiply straight into yta — the pair layout already
    matches the projection's 2-heads-per-128-partitions packing, so the
    odd-head repack DMA of v1 is gone.
  - QKV / v / projection matmul groups are emitted as FILLER STEPS woven
    between attention j-iterations (software pipelining): during the
    ACT-paced attention phase the PE always has an independent GEMM to run.
  - PSUM budget: mm pool 1 bank, sim 2x[128,1024] = 4, yps 2, denom 1 = 8.
"""

import sys

sys.path.insert(0, "/opt/trn_rl_repo")

import numpy as np
import ml_dtypes

BF16 = ml_dtypes.bfloat16

B, T, C = 2, 2048, 1024
H, D = 16, 64
HPC = 4          # heads per core
GC = HPC * D     # head-group channel width (256)
NT = T // 128    # 16 row tiles
NS = T // 512    # 4 query super-tiles

_cached = None


def _build(repeat=1, mode="full"):
    import concourse.bass as bass  # noqa: F401
    import concourse.mybir as mybir
    import concourse.tile as tile
    from concourse import bacc

    f32 = mybir.dt.float32
    bf16 = mybir.dt.bfloat16
    AF = mybir.ActivationFunctionType

    nc = bacc.Bacc(None, target_bir_lowering=False, debug=False)
    xt_d = nc.declare_dram_parameter("xt", [C, T], bf16, isOutput=False)
    wqk_d = nc.declare_dram_parameter("wqk", [C, 2 * GC], bf16, isOutput=False)
    wv_d = nc.declare_dram_parameter("wv", [C, GC], bf16, isOutput=False)
    wp_d = nc.declare_dram_parameter("wp", [GC, C], bf16, isOutput=False)
    bqk_d = nc.declare_dram_parameter("bqk", [2 * GC], f32, isOutput=False)
    bv_d = nc.declare_dram_parameter("bv", [GC], f32, isOutput=False)
    out_d = nc.declare_dram_parameter("out", [T, C], bf16, isOutput=True)

    with tile.TileContext(nc) as tc:
        with (
            tc.tile_pool(name="const", bufs=1) as cpool,
            tc.tile_pool(name="exp", bufs=4) as epool,
            tc.tile_pool(name="ostg", bufs=2) as opool,
            tc.tile_pool(name="smal", bufs=3) as spool,
            tc.tile_pool(name="mm", bufs=2, space="PSUM") as mmp,
            tc.tile_pool(name="sim", bufs=2, space="PSUM") as simp,
            tc.tile_pool(name="yp", bufs=1, space="PSUM") as ypp,
            tc.tile_pool(name="dp", bufs=1, space="PSUM") as dpp,
        ):

            def emit_once():
                # ---- DMAs: first-needed first; wqk m-split so the first
                # QK group only gates on 128KB of weights + 1MB of x ----
                xt_re = xt_d[:].rearrange("(c p) t -> p c t", p=128)
                wqk_re = wqk_d[:].rearrange("(c p) m -> p c m", p=128)
                wqkm = []
                wqkm0 = cpool.tile([128, 8, 128], bf16, tag="wqkm0")
                wqkm.append(wqkm0)
                nc.sync.dma_start(out=wqkm[0][:], in_=wqk_re[:, :, 0:128])
                xa = cpool.tile([128, 4, 512], bf16, tag="xt0a")
                nc.sync.dma_start(out=xa[:], in_=xt_re[:, 0:4, 0:512])
                xb = cpool.tile([128, 4, 512], bf16, tag="xt0b")
                nc.sync.dma_start(out=xb[:], in_=xt_re[:, 4:8, 0:512])
                for m in range(1, 4):
                    wqkm_t = cpool.tile([128, 8, 128], bf16, tag=f"wqkm{m}")
                    nc.sync.dma_start(
                        out=wqkm_t[:], in_=wqk_re[:, :, m * 128 : (m + 1) * 128]
                    )
                    wqkm.append(wqkm_t)
                bqk = cpool.tile([128, 4], f32, tag="bqk")
                nc.sync.dma_start(
                    out=bqk[:], in_=bqk_d[:].rearrange("(m p) -> p m", p=128)
                )
                bv1 = cpool.tile([1, GC], f32, tag="bv1")
                nc.sync.dma_start(
                    out=bv1[:], in_=bv_d[:].rearrange("(o v) -> o v", o=1)
                )
                wv = cpool.tile([128, 8, GC], bf16, tag="wv")
                nc.sync.dma_start(
                    out=wv[:], in_=wv_d[:].rearrange("(c p) m -> p c m", p=128)
                )
                xt_parts = [[(xa, 0), (xb, 4)]]
                for s in range(1, NS):
                    x_s = cpool.tile([128, 8, 512], bf16, tag=f"xt{s}")
                    nc.sync.dma_start(
                        out=x_s[:], in_=xt_re[:, :, s * 512 : (s + 1) * 512]
                    )
                    xt_parts.append([(x_s, 0)])
                wp = cpool.tile([128, 2, C], bf16, tag="wp")
                nc.sync.dma_start(
                    out=wp[:], in_=wp_d[:].rearrange("(j p) n -> p j n", p=128)
                )

                def xslice(s, c):
                    for t_, c0 in xt_parts[s]:
                        if c0 <= c < c0 + 4 or (c0 == 0 and len(xt_parts[s]) == 1):
                            return t_[:, c - c0, :]
                    raise AssertionError

                ones = cpool.tile([1, 128], f32, tag="ones")
                nc.any.memset(ones[:], 1.0)
                onesb = cpool.tile([128, D], bf16, tag="onesb")
                nc.any.memset(onesb[:], 1.0)
                zbias = cpool.tile([128, 1], f32, tag="zbias")
                nc.any.memset(zbias[:], 0.0)

                qkT = cpool.tile([128, 4, T], bf16, tag="qkT")
                bvb = cpool.tile([128, GC], f32, tag="bvb")
                v1 = cpool.tile([128, NT, HPC, D], bf16, tag="v1")
                yta = cpool.tile([128, 2, T], bf16, tag="yta")
                dps = dpp.tile([128, 512], f32, tag="dps")

                # ---- filler step factories (each step ~2 matmuls or one
                # evict; emitted between attention iterations) ----
                def make_steps_qk(s):
                    steps = []
                    for m in range(4):
                        cell = {}

                        def s1(s=s, m=m, cell=cell):
                            ps = mmp.tile([128, 512], f32, tag="mm", name="mmq")
                            cell["ps"] = ps
                            for c in range(4):
                                nc.tensor.matmul(
                                    ps[:],
                                    wqkm[m][:, c, :],
                                    xslice(s, c),
                                    start=(c == 0),
                                    stop=False,
                                )

                        def s2(s=s, m=m, cell=cell):
                            ps = cell["ps"]
                            for c in range(4, 8):
                                nc.tensor.matmul(
                                    ps[:],
                                    wqkm[m][:, c, :],
                                    xslice(s, c),
                                    start=False,
                                    stop=(c == 7),
                                )

                        def s3(s=s, m=m, cell=cell):
                            nc.vector.tensor_scalar_add(
                                qkT[:, m, s * 512 : (s + 1) * 512],
                                cell["ps"][:],
                                bqk[:, m : m + 1],
                            )

                        steps += [s1, s2, s3]
                    return steps

                def make_steps_v(s):
                    steps = []
                    for t in range(s * 4, s * 4 + 4):
                        cell = {}

                        def s1(s=s, t=t, cell=cell):
                            ps = mmp.tile([128, GC], f32, tag="mm", name="mmv")
                            cell["ps"] = ps
                            for c in range(4):
                                nc.tensor.matmul(
                                    ps[:],
                                    xslice(s, c)[
                                        :, (t - 4 * s) * 128 : (t - 4 * s + 1) * 128
                                    ],
                                    wv[:, c, :],
                                    start=(c == 0),
                                    stop=False,
                                )

                        def s2(s=s, t=t, cell=cell):
                            ps = cell["ps"]
                            for c in range(4, 8):
                                nc.tensor.matmul(
                                    ps[:],
                                    xslice(s, c)[
                                        :, (t - 4 * s) * 128 : (t - 4 * s + 1) * 128
                                    ],
                                    wv[:, c, :],
                                    start=False,
                                    stop=(c == 7),
                                )

                        def s3(t=t, cell=cell):
                            nc.vector.tensor_add(
                                v1[:, t, :, :],
                                cell["ps"][:].rearrange("p (l d) -> p l d", d=D),
                                bvb[:].rearrange("p (l d) -> p l d", d=D),
                            )

                        steps += [s1, s2, s3]
                    return steps

                def make_steps_proj(s):
                    steps = []
                    for tt in range(4):
                        t = s * 4 + tt
                        cell = {}

                        def s0(cell=cell):
                            cell["ost"] = opool.tile([128, C], bf16, tag="ost", name="ost")

                        steps.append(s0)
                        for n in range(2):

                            def sA(t=t, n=n, cell=cell):
                                pp = mmp.tile([128, 512], f32, tag="mm", name="mmp")
                                cell["pp"] = pp
                                for j in range(2):
                                    nc.tensor.matmul(
                                        pp[:],
                                        yta[:, j, t * 128 : (t + 1) * 128],
                                        wp[:, j, n * 512 : (n + 1) * 512],
                                        start=(j == 0),
                                        stop=(j == 1),
                                    )

                            def sB(n=n, cell=cell):
                                nc.vector.tensor_copy(
                                    cell["ost"][:, n * 512 : (n + 1) * 512],
                                    cell["pp"][:],
                                )

                            steps += [sA, sB]

                        def sD(t=t, cell=cell):
                            nc.sync.dma_start(
                                out=out_d[t * 128 : (t + 1) * 128, :],
                                in_=cell["ost"][:],
                            )

                        steps.append(sD)
                    return steps

                # ---- attention for one head pair over one super-tile ----
                def emit_att(s, p, gidx, filler):
                    njt = 4 * (s + 1)
                    heads = (2 * p, 2 * p + 1)
                    yps = ypp.tile([128, 512], f32, tag="y")

                    def emit_sim(j):
                        q0 = (j - 4 * s) * 128 if j > 4 * s else 0
                        sp = simp.tile([128, 1024], f32, tag="sim")
                        for h in range(2):
                            po = 0 if mode == "simser" else h * 64
                            nc.tensor.matmul(
                                sp[:, h * 512 + q0 : (h + 1) * 512],
                                qkT[po : po + 64, 2 + p, j * 128 : (j + 1) * 128],
                                qkT[po : po + 64, p, s * 512 + q0 : (s + 1) * 512],
                                start=True,
                                stop=True,
                            )
                        return sp, q0

                    pend = [emit_sim(0)]
                    if njt > 1:
                        pend.append(emit_sim(1))
                    for _ in range(2):
                        if filler:
                            filler.pop(0)()
                    for j in range(njt):
                        sp, q0 = pend.pop(0)
                        ex = epool.tile([128, 1024], bf16, tag="exp")
                        if q0 == 0:
                            nc.scalar.activation(
                                ex[:],
                                sp[:],
                                AF.Exp,
                                bias=zbias[:, 0:1],
                                scale=1.0 / 32.0,
                            )
                        else:
                            # one act over both heads' valid ranges via a
                            # strided [128, 2, 512-q0] AP
                            ex3 = ex[:].rearrange("p (h w) -> p h w", h=2)
                            sp3 = sp[:].rearrange("p (h w) -> p h w", h=2)
                            nc.scalar.activation(
                                ex3[:, :, q0:],
                                sp3[:, :, q0:],
                                AF.Exp,
                                bias=zbias[:, 0:1],
                                scale=1.0 / 32.0,
                            )
                        r = j - 4 * s
                        if 0 <= r < 4:
                            for h in range(2):
                                nc.gpsimd.affine_select(
                                    out=ex[:, h * 512 + q0 : (h + 1) * 512],
                                    in_=ex[:, h * 512 + q0 : (h + 1) * 512],
                                    pattern=[[1, 512 - q0]],
                                    compare_op=mybir.AluOpType.is_ge,
                                    fill=0.0,
                                    base=q0 - r * 128,
                                    channel_multiplier=-1,
                                )
                        if j + 2 < njt:
                            pend.append(emit_sim(j + 2))
                        for h in range(2):
                            nc.tensor.matmul(
                                yps[h * 64 : (h + 1) * 64, q0:],
                                v1[:, j, heads[h], :],
                                ex[:, h * 512 + q0 : (h + 1) * 512],
                                start=(j == 0),
                                stop=(j == njt - 1),
                                skip_group_check=True,
                            )
                        if mode != "nod":
                            for h in range(2):
                                nc.tensor.matmul(
                                    dps[h * 64 : (h + 1) * 64, q0:],
                                    onesb[:],
                                    ex[:, h * 512 + q0 : (h + 1) * 512],
                                    start=(j == 0),
                                    stop=(j == njt - 1),
                                    skip_group_check=True,
                                )
                        if filler:
                            filler.pop(0)()

                    # normalize into yta (pair layout == projection layout):
                    # dps rows h*64..h*64+63 all hold the same denominator
                    # (all-ones M=64 lhsT), so normalization is a plain
                    # elementwise multiply - no partition broadcast needed.
                    if mode in ("nod", "donly"):
                        for h in range(2):
                            nc.vector.tensor_copy(
                                yta[h * 64 : (h + 1) * 64, p, s * 512 : (s + 1) * 512],
                                yps[h * 64 : (h + 1) * 64, :],
                            )
                        return
                    # NOTE: reciprocal_approx_fast misbehaves at base
                    # partition 64, so run one op over all 128 partitions
                    # (every dps row holds a valid denominator).
                    dinv = spool.tile([128, 512], f32, tag="dinv")
                    nc.vector.reciprocal_approx_fast(dinv[:], dps[:])
                    for h in range(2):
                        nc.vector.tensor_mul(
                            yta[h * 64 : (h + 1) * 64, p, s * 512 : (s + 1) * 512],
                            yps[h * 64 : (h + 1) * 64, :],
                            dinv[h * 64 : (h + 1) * 64, :],
                        )

                if mode == "attonly":
                    nc.gpsimd.memset(qkT[:], 0.01)
                    nc.gpsimd.memset(v1[:], 0.5)
                    for s in range(NS):
                        for p in (0, 1):
                            emit_att(s, p, s * 2 + p, [])
                    return
                # ---- prologue: QKV for super-tile 0 + bv broadcast ----
                for st in make_steps_qk(0):
                    st()
                pbv = mmp.tile([128, GC], f32, tag="mm")
                nc.tensor.matmul(
                    pbv[:], ones[:, 0:128], bv1[:], start=True, stop=True
                )
                nc.vector.tensor_copy(bvb[:], pbv[:])
                for st in make_steps_v(0):
                    st()
                if mode == "gemmonly":
                    nc.gpsimd.memset(yta[:], 0.01)
                    for s in range(1, NS):
                        for st in make_steps_qk(s):
                            st()
                        for st in make_steps_v(s):
                            st()
                    for s in range(NS):
                        for st in make_steps_proj(s):
                            st()
                    return

                # ---- main pipeline ----
                for s in range(NS):
                    filler = []
                    if s + 1 < NS:
                        filler += make_steps_qk(s + 1)
                        filler += make_steps_v(s + 1)
                    if s >= 1:
                        filler += make_steps_proj(s - 1)
                    for p in (0, 1):
                        emit_att(s, p, s * 2 + p, filler)
                    while filler:
                        filler.pop(0)()
                for st in make_steps_proj(NS - 1):
                    st()

            for _rep in range(repeat):
                emit_once()

    nc.compile()
    return nc


def _get_nc():
    global _cached
    if _cached is None:
        _cached = _build()
    return _cached


def build_in_maps(inputs):
    x = np.asarray(inputs["x"], dtype=np.float32)
    W_attn = np.asarray(inputs["W_attn"], dtype=np.float32)
    b_attn = np.asarray(inputs["b_attn"], dtype=np.float32)
    W_proj = np.asarray(inputs["W_proj"], dtype=np.float32)

    in_maps = []
    for b in range(B):
        xT = np.ascontiguousarray(x[b].T).astype(BF16)
        for g in range(4):
            c0 = g * GC
            wq = W_attn[:, c0 : c0 + GC]
            wk = W_attn[:, C + c0 : C + c0 + GC]
            wqk = np.ascontiguousarray(np.concatenate([wq, wk], axis=1)).astype(BF16)
            wv = np.ascontiguousarray(
                W_attn[:, 2 * C + c0 : 2 * C + c0 + GC]
            ).astype(BF16)
            wp = np.ascontiguousarray(W_proj[c0 : c0 + GC, :]).astype(BF16)
            bqk = np.concatenate(
                [b_attn[c0 : c0 + GC], b_attn[C + c0 : C + c0 + GC]]
            ).astype(np.float32)
            bv = np.ascontiguousarray(
                b_attn[2 * C + c0 : 2 * C + c0 + GC]
            ).astype(np.float32)
            in_maps.append(
                {"xt": xT, "wqk": wqk, "wv": wv, "wp": wp, "bqk": bqk, "bv": bv}
            )
    return in_maps


def kernel(x, W_attn, b_attn, W_proj, b_proj):
    from concourse.bass_utils import run_bass_kernel_spmd

    b_proj = np.asarray(b_proj, dtype=np.float32)
    nc = _get_nc()
    in_maps = build_in_maps(
        {"x": x, "W_attn": W_attn, "b_attn": b_attn, "W_proj": W_proj}
    )
    res = run_bass_kernel_spmd(nc, in_maps, core_ids=list(range(8)))
    out = np.zeros((B, T, C), dtype=np.float32)
    for b in range(B):
        for g in range(4):
            out[b] += res.results[b * 4 + g]["out"].astype(np.float32)
        out[b] += b_proj
    return out


# revision 16
# speedup vs baseline: 1.5747x; 1.0471x over previous
"""Causal self-attention (B=2, T=2048, C=1024, H=16) on 8 TRN2 NeuronCores.

Sharding: core = b*4 + g  (b in 0..1 batches, g in 0..3 head-groups of 4 heads).
Each core computes QKV for its 4 heads (tensor-parallel columns of W_attn),
full causal attention over T=2048, and a partial projection
y_g @ W_proj[rows_g] -> [T, C].  Host sums the 4 partials per batch and adds
b_proj.

Device layout (v2 — head-pair tile_position packing):
  - x is pre-transposed on host to xT [C, T]; all matmuls contract over
    partitions.  All matmul inputs bf16, accumulation f32.
  - Heads are processed in PAIRS (0,1) and (2,3) per query super-tile.
    sim^T = k.q is computed with K=64 row-tiled matmuls: head A at array
    rows 0-63, head B at rows 64-127 — the PE runs both CONCURRENTLY
    (disjoint row groups), halving sim wall time vs serial K=64 MMs.
  - exp via ScalarE (scale=1/sqrt(C) folded), trimmed to the causally valid
    column range; causal zeroing via gpsimd affine_select on diagonal tiles.
  - attn@v: col-tiled pair — head A -> yps[0:64], head B -> yps[64:128]
    (M=64 each, col groups 0-1 / 2-3, concurrent).  Softmax denominators
    via separate M=1 ones-matmuls into a shared PSUM tile, col-tiled
    pairwise (rows alternate {0,64}/{32,96} between successive pairs to
    avoid WAR stalls on the single denominator bank).
  - normalize: DVE reciprocal of the denominator rows, gpsimd
    partition_broadcast (DMA partition-hop when the row isn't physical
    partition 0), DVE multiply straight into yta — the pair layout already
    matches the projection's 2-heads-per-128-partitions packing, so the
    odd-head repack DMA of v1 is gone.
  - QKV / v / projection matmul groups are emitted as FILLER STEPS woven
    between attention j-iterations (software pipelining): during the
    ACT-paced attention phase the PE always has an independent GEMM to run.
  - PSUM budget: mm pool 1 bank, sim 2x[128,1024] = 4, yps 2, denom 1 = 8.
"""

import sys

sys.path.insert(0, "/opt/trn_rl_repo")

import numpy as np
import ml_dtypes

BF16 = ml_dtypes.bfloat16

B, T, C = 2, 2048, 1024
H, D = 16, 64
HPC = 4          # heads per core
GC = HPC * D     # head-group channel width (256)
NT = T // 128    # 16 row tiles
NS = T // 512    # 4 query super-tiles

_cached = None


def _build(repeat=1, mode="full"):
    import concourse.bass as bass  # noqa: F401
    import concourse.mybir as mybir
    import concourse.tile as tile
    from concourse import bacc

    f32 = mybir.dt.float32
    bf16 = mybir.dt.bfloat16
    AF = mybir.ActivationFunctionType

    nc = bacc.Bacc(None, target_bir_lowering=False, debug=False)
    xt_d = nc.declare_dram_parameter("xt", [C, T], bf16, isOutput=False)
    wqk_d = nc.declare_dram_parameter("wqk", [C, 2 * GC], bf16, isOutput=False)
    wv_d = nc.declare_dram_parameter("wv", [C, GC], bf16, isOutput=False)
    wp_d = nc.declare_dram_parameter("wp", [GC, C], bf16, isOutput=False)
    bqk_d = nc.declare_dram_parameter("bqk", [2 * GC], f32, isOutput=False)
    bv_d = nc.declare_dram_parameter("bv", [GC], f32, isOutput=False)
    out_d = nc.declare_dram_parameter("out", [T, C], bf16, isOutput=True)

    with tile.TileContext(nc) as tc:
        with (
            tc.tile_pool(name="const", bufs=1) as cpool,
            tc.tile_pool(name="exp", bufs=4) as epool,
            tc.tile_pool(name="ostg", bufs=2) as opool,
            tc.tile_pool(name="smal", bufs=3) as spool,
            tc.tile_pool(name="mm", bufs=2, space="PSUM") as mmp,
            tc.tile_pool(name="sim", bufs=2, space="PSUM") as simp,
            tc.tile_pool(name="yp", bufs=1, space="PSUM") as ypp,
            tc.tile_pool(name="dp", bufs=1, space="PSUM") as dpp,
        ):

            def emit_once():
                # ---- DMAs: first-needed first; wqk m-split so the first
                # QK group only gates on 128KB of weights + 1MB of x ----
                xt_re = xt_d[:].rearrange("(c p) t -> p c t", p=128)
                wqk_re = wqk_d[:].rearrange("(c p) m -> p c m", p=128)
                wqkm = []
                wqkm0 = cpool.tile([128, 8, 128], bf16, tag="wqkm0")
                wqkm.append(wqkm0)
                nc.sync.dma_start(out=wqkm[0][:], in_=wqk_re[:, :, 0:128])
                xa = cpool.tile([128, 4, 512], bf16, tag="xt0a")
                nc.sync.dma_start(out=xa[:], in_=xt_re[:, 0:4, 0:512])
                xb = cpool.tile([128, 4, 512], bf16, tag="xt0b")
                nc.sync.dma_start(out=xb[:], in_=xt_re[:, 4:8, 0:512])
                for m in range(1, 4):
                    wqkm_t = cpool.tile([128, 8, 128], bf16, tag=f"wqkm{m}")
                    nc.sync.dma_start(
                        out=wqkm_t[:], in_=wqk_re[:, :, m * 128 : (m + 1) * 128]
                    )
                    wqkm.append(wqkm_t)
                bqk = cpool.tile([128, 4], f32, tag="bqk")
                nc.sync.dma_start(
                    out=bqk[:], in_=bqk_d[:].rearrange("(m p) -> p m", p=128)
                )
                bv1 = cpool.tile([1, GC], f32, tag="bv1")
                nc.sync.dma_start(
                    out=bv1[:], in_=bv_d[:].rearrange("(o v) -> o v", o=1)
                )
                wv = cpool.tile([128, 8, GC], bf16, tag="wv")
                nc.sync.dma_start(
                    out=wv[:], in_=wv_d[:].rearrange("(c p) m -> p c m", p=128)
                )
                xt_parts = [[(xa, 0), (xb, 4)]]
                for s in range(1, NS):
                    x_s = cpool.tile([128, 8, 512], bf16, tag=f"xt{s}")
                    nc.sync.dma_start(
                        out=x_s[:], in_=xt_re[:, :, s * 512 : (s + 1) * 512]
                    )
                    xt_parts.append([(x_s, 0)])
                wp = cpool.tile([128, 2, C], bf16, tag="wp")
                nc.sync.dma_start(
                    out=wp[:], in_=wp_d[:].rearrange("(j p) n -> p j n", p=128)
                )

                def xslice(s, c):
                    for t_, c0 in xt_parts[s]:
                        if c0 <= c < c0 + 4 or (c0 == 0 and len(xt_parts[s]) == 1):
                            return t_[:, c - c0, :]
                    raise AssertionError

                ones = cpool.tile([1, 128], f32, tag="ones")
                nc.any.memset(ones[:], 1.0)
                onesb = cpool.tile([128, D], bf16, tag="onesb")
                nc.any.memset(onesb[:], 1.0)
                zbias = cpool.tile([128, 1], f32, tag="zbias")
                nc.any.memset(zbias[:], 0.0)

                qkT = cpool.tile([128, 4, T], bf16, tag="qkT")
                bvb = cpool.tile([128, GC], f32, tag="bvb")
                v1 = cpool.tile([128, NT, HPC, D], bf16, tag="v1")
                yta = cpool.tile([128, 2, T], bf16, tag="yta")
                dps = dpp.tile([128, 512], f32, tag="dps")

                # ---- filler step factories (each step ~2 matmuls or one
                # evict; emitted between attention iterations) ----
                def make_steps_qk(s):
                    steps = []
                    for m in range(4):
                        cell = {}

                        def s1(s=s, m=m, cell=cell):
                            ps = mmp.tile([128, 512], f32, tag="mm", name="mmq")
                            cell["ps"] = ps
                            for c in range(4):
                                nc.tensor.matmul(
                                    ps[:],
                                    wqkm[m][:, c, :],
                                    xslice(s, c),
                                    start=(c == 0),
                                    stop=False,
                                )

                        def s2(s=s, m=m, cell=cell):
                            ps = cell["ps"]
                            for c in range(4, 8):
                                nc.tensor.matmul(
                                    ps[:],
                                    wqkm[m][:, c, :],
                                    xslice(s, c),
                                    start=False,
                                    stop=(c == 7),
                                )

                        def s3(s=s, m=m, cell=cell):
                            nc.vector.tensor_scalar_add(
                                qkT[:, m, s * 512 : (s + 1) * 512],
                                cell["ps"][:],
                                bqk[:, m : m + 1],
                            )

                        steps += [s1, s2, s3]
                    return steps

                def make_steps_v(s):
                    steps = []
                    for t in range(s * 4, s * 4 + 4):
                        cell = {}

                        def s1(s=s, t=t, cell=cell):
                            ps = mmp.tile([128, GC], f32, tag="mm", name="mmv")
                            cell["ps"] = ps
                            for c in range(4):
                                nc.tensor.matmul(
                                    ps[:],
                                    xslice(s, c)[
                                        :, (t - 4 * s) * 128 : (t - 4 * s + 1) * 128
                                    ],
                                    wv[:, c, :],
                                    start=(c == 0),
                                    stop=False,
                                )

                        def s2(s=s, t=t, cell=cell):
                            ps = cell["ps"]
                            for c in range(4, 8):
                                nc.tensor.matmul(
                                    ps[:],
                                    xslice(s, c)[
                                        :, (t - 4 * s) * 128 : (t - 4 * s + 1) * 128
                                    ],
                                    wv[:, c, :],
                                    start=False,
                                    stop=(c == 7),
                                )

                        def s3(t=t, cell=cell):
                            nc.vector.tensor_add(
                                v1[:, t, :, :],
                                cell["ps"][:].rearrange("p (l d) -> p l d", d=D),
                                bvb[:].rearrange("p (l d) -> p l d", d=D),
                            )

                        steps += [s1, s2, s3]
                    return steps

                def make_steps_proj(s):
                    steps = []
                    for tt in range(4):
                        t = s * 4 + tt
                        cell = {}

                        def s0(cell=cell):
                            cell["ost"] = opool.tile([128, C], bf16, tag="ost", name="ost")

                        steps.append(s0)
                        for n in range(2):

                            def sA(t=t, n=n, cell=cell):
                                pp = mmp.tile([128, 512], f32, tag="mm", name="mmp")
                                cell["pp"] = pp
                                for j in range(2):
                                    nc.tensor.matmul(
                                        pp[:],
                                        yta[:, j, t * 128 : (t + 1) * 128],
                                        wp[:, j, n * 512 : (n + 1) * 512],
                                        start=(j == 0),
                                        stop=(j == 1),
                                    )

                            def sB(n=n, cell=cell):
                                nc.vector.tensor_copy(
                                    cell["ost"][:, n * 512 : (n + 1) * 512],
                                    cell["pp"][:],
                                )

                            steps += [sA, sB]

                        def sD(t=t, cell=cell):
                            nc.sync.dma_start(
                                out=out_d[t * 128 : (t + 1) * 128, :],
                                in_=cell["ost"][:],
                            )

                        steps.append(sD)
                    return steps

                # ---- attention for one head pair over one super-tile ----
                def emit_att(s, p, gidx, filler):
                    njt = 4 * (s + 1)
                    heads = (2 * p, 2 * p + 1)
                    yps = ypp.tile([128, 512], f32, tag="y")

                    def emit_sim(j):
                        q0 = (j - 4 * s) * 128 if j > 4 * s else 0
                        sp = simp.tile([128, 1024], f32, tag="sim")
                        for h in range(2):
                            po = 0 if mode == "simser" else h * 64
                            nc.tensor.matmul(
                                sp[:, h * 512 + q0 : (h + 1) * 512],
                                qkT[po : po + 64, 2 + p, j * 128 : (j + 1) * 128],
                                qkT[po : po + 64, p, s * 512 + q0 : (s + 1) * 512],
                                start=True,
                                stop=True,
                            )
                        return sp, q0

                    pend = [emit_sim(0)]
                    if njt > 1:
                        pend.append(emit_sim(1))
                    for _ in range(2):
                        if filler:
                            filler.pop(0)()
                    for j in range(njt):
                        sp, q0 = pend.pop(0)
                        ex = epool.tile([128, 1024], bf16, tag="exp")
                        if q0 == 0:
                            nc.scalar.activation(
                                ex[:],
                                sp[:],
                                AF.Exp,
                                bias=zbias[:, 0:1],
                                scale=1.0 / 32.0,
                            )
                        else:
                            # one act over both heads' valid ranges via a
                            # strided [128, 2, 512-q0] AP
                            ex3 = ex[:].rearrange("p (h w) -> p h w", h=2)
                            sp3 = sp[:].rearrange("p (h w) -> p h w", h=2)
                            nc.scalar.activation(
                                ex3[:, :, q0:],
                                sp3[:, :, q0:],
                                AF.Exp,
                                bias=zbias[:, 0:1],
                                scale=1.0 / 32.0,
                            )
                        r = j - 4 * s
                        if 0 <= r < 4:
                            for h in range(2):
                                nc.gpsimd.affine_select(
                                    out=ex[:, h * 512 + q0 : (h + 1) * 512],
                                    in_=ex[:, h * 512 + q0 : (h + 1) * 512],
                                    pattern=[[1, 512 - q0]],
                                    compare_op=mybir.AluOpType.is_ge,
                                    fill=0.0,
                                    base=q0 - r * 128,
                                    channel_multiplier=-1,
                                )
                        for _ in range(2):
                            if filler:
                                filler.pop(0)()
                        if j + 2 < njt:
                            pend.append(emit_sim(j + 2))
                        for h in range(2):
                            nc.tensor.matmul(
                                yps[h * 64 : (h + 1) * 64, q0:],
                                v1[:, j, heads[h], :],
                                ex[:, h * 512 + q0 : (h + 1) * 512],
                                start=(j == 0),
                                stop=(j == njt - 1),
                                skip_group_check=True,
                            )
                        if mode != "nod":
                            for h in range(2):
                                nc.tensor.matmul(
                                    dps[h * 64 : (h + 1) * 64, q0:],
                                    onesb[:],
                                    ex[:, h * 512 + q0 : (h + 1) * 512],
                                    start=(j == 0),
                                    stop=(j == njt - 1),
                                    skip_group_check=True,
                                )

                    # normalize into yta (pair layout == projection layout):
                    # dps rows h*64..h*64+63 all hold the same denominator
                    # (all-ones M=64 lhsT), so normalization is a plain
                    # elementwise multiply - no partition broadcast needed.
                    if mode in ("nod", "donly"):
                        for h in range(2):
                            nc.vector.tensor_copy(
                                yta[h * 64 : (h + 1) * 64, p, s * 512 : (s + 1) * 512],
                                yps[h * 64 : (h + 1) * 64, :],
                            )
                        return
                    # NOTE: reciprocal_approx_fast misbehaves at base
                    # partition 64, so run one op over all 128 partitions
                    # (every dps row holds a valid denominator).
                    dinv = spool.tile([128, 512], f32, tag="dinv")
                    nc.vector.reciprocal_approx_fast(dinv[:], dps[:])
                    for h in range(2):
                        nc.vector.tensor_mul(
                            yta[h * 64 : (h + 1) * 64, p, s * 512 : (s + 1) * 512],
                            yps[h * 64 : (h + 1) * 64, :],
                            dinv[h * 64 : (h + 1) * 64, :],
                        )

                if mode == "attonly":
                    nc.gpsimd.memset(qkT[:], 0.01)
                    nc.gpsimd.memset(v1[:], 0.5)
                    for s in range(NS):
                        for p in (0, 1):
                            emit_att(s, p, s * 2 + p, [])
                    return
                # ---- prologue: QKV for super-tile 0 + bv broadcast ----
                for st in make_steps_qk(0):
                    st()
                pbv = mmp.tile([128, GC], f32, tag="mm")
                nc.tensor.matmul(
                    pbv[:], ones[:, 0:128], bv1[:], start=True, stop=True
                )
                nc.vector.tensor_copy(bvb[:], pbv[:])
                for st in make_steps_v(0):
                    st()
                if mode == "gemmonly":
                    nc.gpsimd.memset(yta[:], 0.01)
                    for s in range(1, NS):
                        for st in make_steps_qk(s):
                            st()
                        for st in make_steps_v(s):
                            st()
                    for s in range(NS):
                        for st in make_steps_proj(s):
                            st()
                    return

                # ---- main pipeline ----
                for s in range(NS):
                    filler = []
                    if s + 1 < NS:
                        filler += make_steps_qk(s + 1)
                        filler += make_steps_v(s + 1)
                    if s >= 1:
                        filler += make_steps_proj(s - 1)
                    for p in (0, 1):
                        emit_att(s, p, s * 2 + p, filler)
                    while filler:
                        filler.pop(0)()
                for st in make_steps_proj(NS - 1):
                    st()

            for _rep in range(repeat):
                emit_once()

    nc.compile()
    return nc


def _get_nc():
    global _cached
    if _cached is None:
        _cached = _build()
    return _cached


def build_in_maps(inputs):
    x = np.asarray(inputs["x"], dtype=np.float32)
    W_attn = np.asarray(inputs["W_attn"], dtype=np.float32)
    b_attn = np.asarray(inputs["b_attn"], dtype=np.float32)
    W_proj = np.asarray(inputs["W_proj"], dtype=np.float32)

    in_maps = []
    for b in range(B):
        xT = np.ascontiguousarray(x[b].T).astype(BF16)
        for g in range(4):
            c0 = g * GC
            wq = W_attn[:, c0 : c0 + GC]
            wk = W_attn[:, C + c0 : C + c0 + GC]
            wqk = np.ascontiguousarray(np.concatenate([wq, wk], axis=1)).astype(BF16)
            wv = np.ascontiguousarray(
                W_attn[:, 2 * C + c0 : 2 * C + c0 + GC]
            ).astype(BF16)
            wp = np.ascontiguousarray(W_proj[c0 : c0 + GC, :]).astype(BF16)
            bqk = np.concatenate(
                [b_attn[c0 : c0 + GC], b_attn[C + c0 : C + c0 + GC]]
            ).astype(np.float32)
            bv = np.ascontiguousarray(
                b_attn[2 * C + c0 : 2 * C + c0 + GC]
            ).astype(np.float32)
            in_maps.append(
                {"xt": xT, "wqk": wqk, "wv": wv, "wp": wp, "bqk": bqk, "bv": bv}
            )
    return in_maps


def kernel(x, W_attn, b_attn, W_proj, b_proj):
    from concourse.bass_utils import run_bass_kernel_spmd

    b_proj = np.asarray(b_proj, dtype=np.float32)
    nc = _get_nc()
    in_maps = build_in_maps(
        {"x": x, "W_attn": W_attn, "b_attn": b_attn, "W_proj": W_proj}
    )
    res = run_bass_kernel_spmd(nc, in_maps, core_ids=list(range(8)))
    out = np.zeros((B, T, C), dtype=np.float32)
    for b in range(B):
        for g in range(4):
            out[b] += res.results[b * 4 + g]["out"].astype(np.float32)
        out[b] += b_proj
    return out
